# revision 8
# baseline (speedup 1.0000x reference)
"""BertSelfAttention Trainium2 kernel.

Full inputs in, full output out. Sharding: 8 cores = (batch b in {0,1}) x
(head-group hg in {0..3}); each core computes 4 heads of one batch and
produces the output feature slice out[b, :, hg*256:(hg+1)*256].

Per-core device program (all cores run the same NEFF, SPMD):
  xT [1024, 2048]      hidden_states[b].T
  QT/KT computed transposed [d, s] (fp32r matmuls), stored fp16 with bias
  V computed [s, d] fp16, rows scaled by exp(mask), plus a per-head
    ones*exp(mask) column so the ctx matmul also yields softmax row sums
  scoresT [k, q] tiles via fp16 matmuls (two heads on the two PE
    row-halves via tile_position)
  exp on ACT directly from PSUM (scale=1/8, bias=-4 folded in)
  ctx[q, d] = expT.T @ [V|em] accumulated over 16 k-tiles, then
    per-partition normalize (reciprocal of row sum) + V-bias add on DVE.

All PE instructions are chained with nosync deps in a hand-balanced
order (scores batches / ctx groups / projection filler) so the PE never
idles long enough for the HAM clock gate to re-throttle it to 1.2 GHz.
"""

import numpy as np

B = 2
S = 2048
H = 1024
NH = 16
HD = 64

NCORES = 8
HPC = 4          # heads per core
DS = HPC * HD    # 256 output dims per core
FT = H // 128    # 8 f-tiles (contraction tiles for projections)
KT = S // 128    # 16 key tiles
ST = S // 128    # 16 s-tiles of V
QB = 4           # q blocks of 512
QBS = 512
VW = HPC * (HD + 1)  # 260: V columns + one em column per head

EXP_BIAS = -4.0  # uniform shift inside exp; cancels in softmax, guards fp16

_CACHE = {}


def _build_program(split_waits=True):
    import concourse.bass as bass
    import concourse.mybir as mybir
    import concourse.tile as tile
    from concourse.tile_rust import add_dep_helper
    from concourse.vector_clock import ScopedClock

    f32 = mybir.dt.float32
    f32r = mybir.dt.float32r
    f16 = mybir.dt.float16
    AF = mybir.ActivationFunctionType
    OP = mybir.AluOpType

    class SplitDrainTileContext(tile.TileContext):
        """The walrus build here rejects instructions with more than one
        sync wait ("Too many sync wait commands"); hoist excess waits onto
        preceding same-engine NOPs."""

        MAX_WAITS_PER_DRAIN = 1
        split_waits_enabled = True

        def _drain_and_barrier(self, tick_clock, wait_clock):
            drain_inst = self.nc.sync.drain()
            wait_clock.add_sem_waits(
                drain_inst.ins, ScopedClock({None: tick_clock.global_clock})
            )
            self.nc.all_engine_barrier()
            assert self.sems is not None
            popped = self.nc._tile_sem_poison_stack.pop()
            assert popped is self._sem_poison
            self.nc.clear_and_free_semaphores(list(self.sems.allocated().values()))
            self.nc.all_engine_barrier()
            if self.split_waits_enabled:
                self._split_multi_waits()

        def _split_multi_waits(self):
            k = self.MAX_WAITS_PER_DRAIN
            nc = self.nc
            for bb in nc.bb_map.values():
                il = bb.bb.instructions
                new = []
                for inst in il:
                    si = getattr(inst, "sync_info", None)
                    waits = list(si.on_wait) if si is not None and si.on_wait else []
                    if len(waits) > k:
                        for j in range(0, len(waits) - k, k):
                            nop = mybir.InstNoOp(
                                name=nc.get_next_instruction_name(),
                                engine=inst.engine,
                                sync_info=mybir.SyncInfo(
                                    on_wait=waits[j : j + k], on_update=[]
                                ),
                                bass_nofuse=True,
                            )
                            new.append(nop)
                        inst.sync_info = mybir.SyncInfo(
                            on_wait=waits[len(waits) - k :],
                            on_update=list(si.on_update) if si.on_update else [],
                        )
                    new.append(inst)
                il[:] = new

    nc = bass.Bass("TRN2", target_bir_lowering=False, debug=False,
                   num_devices=NCORES)

    xT_d = nc.dram_tensor("xT", [H, S], f32, kind="ExternalInput")
    wqT_d = nc.dram_tensor("wqT", [H, DS], f32, kind="ExternalInput")
    wkT_d = nc.dram_tensor("wkT", [H, DS], f32, kind="ExternalInput")
    wvT_d = nc.dram_tensor("wvT", [H, VW], f32, kind="ExternalInput")
    bq_d = nc.dram_tensor("bq", [2, 128, 1], f32, kind="ExternalInput")
    bk_d = nc.dram_tensor("bk", [2, 128, 1], f32, kind="ExternalInput")
    bvb_d = nc.dram_tensor("bvb", [128, DS], f32, kind="ExternalInput")
    em_d = nc.dram_tensor("em", [128, KT], f32, kind="ExternalInput")
    out_d = nc.dram_tensor("out", [S, DS], f32, kind="ExternalOutput")

    SplitDrainTileContext.split_waits_enabled = split_waits
    with SplitDrainTileContext(nc) as tc:
        from contextlib import ExitStack

        with ExitStack() as ctx:
            const = ctx.enter_context(tc.tile_pool(name="const", bufs=1))
            qk = ctx.enter_context(tc.tile_pool(name="qk", bufs=1))
            vp = ctx.enter_context(tc.tile_pool(name="vp", bufs=1))
            epool = ctx.enter_context(tc.tile_pool(name="epool", bufs=1))
            opool = ctx.enter_context(tc.tile_pool(name="opool", bufs=1))
            rpool = ctx.enter_context(tc.tile_pool(name="rpool", bufs=1))

            # ---- constants ----
            bq_sb = [const.tile([128, 1], f32, tag=f"bq{m}", bufs=1,
                                name=f"bq_sb{m}") for m in range(2)]
            bk_sb = [const.tile([128, 1], f32, tag=f"bk{m}", bufs=1,
                                name=f"bk_sb{m}") for m in range(2)]
            for m in range(2):
                nc.sync.dma_start(bq_sb[m][:], bq_d.ap()[m])
                nc.sync.dma_start(bk_sb[m][:], bk_d.ap()[m])
            bvb_sb = const.tile([128, DS], f32, tag="bvb", bufs=1, name="bvb_sb")
            nc.sync.dma_start(bvb_sb[:], bvb_d.ap())
            em_sb = const.tile([128, KT], f32, tag="em", bufs=1, name="em_sb")
            nc.sync.dma_start(em_sb[:], em_d.ap())
            ebias = const.tile([128, 1], f32, tag="ebias", bufs=1, name="ebias")
            nc.vector.memset(ebias[:], EXP_BIAS)
            # warm the ACT exp table while DMAs run
            warm = const.tile([128, 1], f32, tag="warm", bufs=1, name="warm")
            nc.scalar.activation(warm[:], ebias[:], AF.Exp)

            # ---- persistent activations ----
            qt = [qk.tile([128, S], f16, tag=f"qt{m}", bufs=1, name=f"qt{m}")
                  for m in range(2)]
            kt_sb = [qk.tile([128, S], f16, tag=f"kt{m}", bufs=1, name=f"kt{m}")
                     for m in range(2)]
            vones = [vp.tile([128, VW], f16, tag=f"v{st}", bufs=1,
                             name=f"vones{st}") for st in range(ST)]

            # ---- input DMAs (xT + m0-needed weights first) ----
            xw = ctx.enter_context(tc.tile_pool(name="xw", bufs=1))
            xt = [xw.tile([128, S], f32, tag=f"xt{ft}", bufs=1, name=f"xt{ft}")
                  for ft in range(FT)]
            wq_sb = [xw.tile([128, DS], f32, tag=f"wq{ft}", bufs=1,
                             name=f"wq{ft}") for ft in range(FT)]
            wk_sb = [xw.tile([128, DS], f32, tag=f"wk{ft}", bufs=1,
                             name=f"wk{ft}") for ft in range(FT)]
            wv_sb = [xw.tile([128, VW], f32, tag=f"wv{ft}", bufs=1,
                             name=f"wv{ft}") for ft in range(FT)]
            for ft in range(FT):
                fs = slice(ft * 128, (ft + 1) * 128)
                nc.sync.dma_start(
                    xt[ft][:].bitcast(f32r), xT_d.ap()[fs, :].bitcast(f32r))
                nc.sync.dma_start(
                    wq_sb[ft][:].bitcast(f32r), wqT_d.ap()[fs, :].bitcast(f32r))
                nc.sync.dma_start(
                    wk_sb[ft][:].bitcast(f32r), wkT_d.ap()[fs, :].bitcast(f32r))
            for ft in range(FT):
                fs = slice(ft * 128, (ft + 1) * 128)
                nc.sync.dma_start(
                    wv_sb[ft][:].bitcast(f32r), wvT_d.ap()[fs, :].bitcast(f32r))

            # ---- PSUM pools: proj 1 + scores 2x3 + ctx 1 = 8 banks ----
            ps_pj = ctx.enter_context(
                tc.tile_pool(name="ps_pj", bufs=1, space="PSUM"))
            ps_sc = ctx.enter_context(
                tc.tile_pool(name="ps_sc", bufs=2, space="PSUM"))
            ps_cx = ctx.enter_context(
                tc.tile_pool(name="ps_cx", bufs=1, space="PSUM"))

            # ---- PE chain: every PE matmul nosync-depends on the previous
            # one, so the scheduler keeps our hand-balanced order and the
            # PE never idles into a HAM re-throttle window.
            pe_prev = [None]

            def mm(*args, **kw):
                inst = nc.tensor.matmul(*args, **kw)
                if pe_prev[0] is not None:
                    add_dep_helper(inst.ins, pe_prev[0].ins, sync=False,
                                   reason="pe-chain")
                pe_prev[0] = inst
                return inst

            # ---- work units (each emits PE work + its evictions) ----
            def qk_proj_block(w_sb, bias_sb, dst, m, nb):
                ns = slice(nb * QBS, (nb + 1) * QBS)
                ps = ps_pj.tile([128, QBS], f32, tag="pj", name="pspj")
                for ft in range(FT):
                    mm(ps[:],
                       w_sb[ft][:, m * 128:(m + 1) * 128].bitcast(f32r),
                       xt[ft][:, ns].bitcast(f32r),
                       start=(ft == 0), stop=(ft == FT - 1))
                nc.vector.tensor_scalar_add(dst[:, ns], ps[:], bias_sb[:])

            def v_proj_block(st):
                ss = slice(st * 128, (st + 1) * 128)
                ps = ps_pj.tile([128, QBS], f32, tag="pj", name="pspjv")
                for ft in range(FT):
                    mm(ps[:, 0:VW],
                       xt[ft][:, ss].bitcast(f32r),
                       wv_sb[ft][:].bitcast(f32r),
                       start=(ft == 0), stop=(ft == FT - 1))
                nc.vector.tensor_scalar_mul(
                    vones[st][:], ps[:, 0:VW], em_sb[:, st:st + 1])
                for hh in range(HPC):
                    c = hh * (HD + 1) + HD
                    nc.vector.tensor_copy(
                        vones[st][:, c:c + 1], em_sb[:, st:st + 1])

            BATCHES = [(0, 3), (3, 3), (6, 3), (9, 3), (12, 3), (15, 1)]

            def scores_batch(hp, qb, eA, eB, k0, nk):
                qs = slice(qb * QBS, (qb + 1) * QBS)
                psA = ps_sc.tile([128, 3 * QBS], f32, tag="sc", name="pscA")
                psB = ps_sc.tile([128, 3 * QBS], f32, tag="sc", name="pscB")
                for j in range(nk):
                    ktile = k0 + j
                    ks = slice(ktile * 128, (ktile + 1) * 128)
                    js = slice(j * QBS, (j + 1) * QBS)
                    mm(psA[:, js], kt_sb[hp][0:64, ks], qt[hp][0:64, qs],
                       tile_position=(0, 0))
                    mm(psB[:, js], kt_sb[hp][64:128, ks], qt[hp][64:128, qs],
                       tile_position=(64, 0))
                w = nk * QBS
                es = slice(k0 * QBS, k0 * QBS + w)
                nc.scalar.activation(eA[:, es], psA[:, 0:w], AF.Exp,
                                     bias=ebias[:], scale=0.125)
                nc.scalar.activation(eB[:, es], psB[:, 0:w], AF.Exp,
                                     bias=ebias[:], scale=0.125)

            def ctx_unit(hp, qb, a, e, ots):
                """One head's 4 q-tiles of ctx for (hp, qb)."""
                hh = 2 * hp + a
                qtile0 = qb * 4
                cpsb = ps_cx.tile([128, 4 * (HD + 1)], f32, tag="cx",
                                  name="cps")
                for qq in range(4):
                    cps = cpsb[:, qq * (HD + 1):(qq + 1) * (HD + 1)]
                    for ktile in range(KT):
                        lo = ktile * QBS + qq * 128
                        mm(cps,
                           e[:, lo:lo + 128],
                           vones[ktile][:, hh * (HD + 1):(hh + 1) * (HD + 1)],
                           start=(ktile == 0), stop=(ktile == KT - 1))
                    if a == 0:
                        ot = opool.tile([128, 128], f32, tag="ot", bufs=4,
                                        name="ot")
                        ots.append(ot)
                    else:
                        ot = ots[qq]
                    r = rpool.tile([128, 1], f32, tag="r", bufs=4, name="r")
                    nc.vector.reciprocal(r[:], cps[:, HD:HD + 1])
                    nc.vector.scalar_tensor_tensor(
                        ot[:, a * 64:(a + 1) * 64],
                        cps[:, 0:HD], r[:],
                        bvb_sb[:, hh * HD:(hh + 1) * HD],
                        op0=OP.mult, op1=OP.add)
                    if a == 1:
                        qt_idx = qtile0 + qq
                        nc.sync.dma_start(
                            out_d.ap()[qt_idx * 128:(qt_idx + 1) * 128,
                                       hp * 128:(hp + 1) * 128],
                            ot[:])

            def ctx_units(prev_state):
                hp, qb, eA, eB = prev_state
                shared_ots = []
                return [
                    lambda: ctx_unit(hp, qb, 0, eA, shared_ots),
                    lambda: ctx_unit(hp, qb, 1, eB, shared_ots),
                ]

            # ---- emission schedule ----
            # m0 Q/K projection, pipelined by 512-col blocks
            for nb in range(QB):
                qk_proj_block(wq_sb, bq_sb[0], qt[0], 0, nb)
                qk_proj_block(wk_sb, bk_sb[0], kt_sb[0], 0, nb)

            # filler units per attention iteration index 0..7
            def m1_block(nb):
                qk_proj_block(wq_sb, bq_sb[1], qt[1], 1, nb)
                qk_proj_block(wk_sb, bk_sb[1], kt_sb[1], 1, nb)

            fillers = {
                0: [lambda st=st: v_proj_block(st) for st in range(10)],
                1: [lambda st=st: v_proj_block(st) for st in range(10, ST)],
                2: [lambda nb=nb: m1_block(nb) for nb in (0, 1)],
                3: [lambda nb=nb: m1_block(nb) for nb in (2, 3)],
            }

            prev = None
            for it in range(8):
                hp, qb = divmod(it, QB)
                eA = epool.tile([128, KT * QBS], f16, tag="eA", bufs=2,
                                name="eA")
                eB = epool.tile([128, KT * QBS], f16, tag="eB", bufs=2,
                                name="eB")
                # other units: filler first (V needed by upcoming ctx),
                # then ctx of the previous iteration
                others = list(fillers.get(it, []))
                if prev is not None:
                    others.extend(ctx_units(prev))
                batches = [
                    (lambda k0=k0, nk=nk: scores_batch(hp, qb, eA, eB, k0, nk))
                    for (k0, nk) in BATCHES
                ]
                # two score batches up front so ACT has work, then spread
                # the other units between the remaining batches
                batches[0]()
                batches[1]()
                n_slots = len(batches) - 1
                per = [len(others) // n_slots] * n_slots
                for i in range(len(others) % n_slots):
                    per[i] += 1
                oi = 0
                for si in range(2, len(batches)):
                    batches[si]()
                    for _ in range(per[si - 2]):
                        others[oi]()
                        oi += 1
                for _ in range(per[-1]):
                    others[oi]()
                    oi += 1
                prev = (hp, qb, eA, eB)
            for u in ctx_units(prev):
                u()

    return nc


def _get_program(split_waits=True):
    key = ("nc", split_waits)
    if key not in _CACHE:
        _CACHE[key] = _build_program(split_waits)
    return _CACHE[key]


def _make_in_maps(hidden_states, attention_mask, Wq, bq, Wk, bk, Wv, bv):
    hidden = np.ascontiguousarray(np.asarray(hidden_states, dtype=np.float32))
    mask = np.asarray(attention_mask, dtype=np.float32)
    Wq = np.asarray(Wq, dtype=np.float32)
    Wk = np.asarray(Wk, dtype=np.float32)
    Wv = np.asarray(Wv, dtype=np.float32)
    bq = np.asarray(bq, dtype=np.float32)
    bk = np.asarray(bk, dtype=np.float32)
    bv = np.asarray(bv, dtype=np.float32)

    WqT = Wq.T  # [in, out]
    WkT = Wk.T
    WvT = Wv.T

    in_maps = []
    for c in range(NCORES):
        b, hg = divmod(c, HPC)
        cols = slice(hg * DS, (hg + 1) * DS)
        xT = np.ascontiguousarray(hidden[b].T)
        wqT = np.ascontiguousarray(WqT[:, cols])
        wkT = np.ascontiguousarray(WkT[:, cols])
        wv_base = WvT[:, cols]
        wvT = np.zeros((H, VW), np.float32)
        for hh in range(HPC):
            wvT[:, hh * (HD + 1):hh * (HD + 1) + HD] = \
                wv_base[:, hh * HD:(hh + 1) * HD]
        bq_c = np.ascontiguousarray(bq[cols].reshape(2, 128, 1))
        bk_c = np.ascontiguousarray(bk[cols].reshape(2, 128, 1))
        bvb = np.ascontiguousarray(np.tile(bv[cols][None, :], (128, 1)))
        em = np.ascontiguousarray(
            np.exp(mask[b, 0, 0, :]).reshape(KT, 128).T.astype(np.float32))
        in_maps.append({
            "xT": xT, "wqT": wqT, "wkT": wkT, "wvT": wvT,
            "bq": bq_c, "bk": bk_c, "bvb": bvb, "em": em,
        })
    return in_maps


def _assemble(results):
    out = np.empty((B, S, H), np.float32)
    for c in range(NCORES):
        b, hg = divmod(c, HPC)
        out[b][:, hg * DS:(hg + 1) * DS] = results[c]["out"]
    return out


def _run(in_maps, trace=False):
    from concourse.bass_utils import run_bass_kernel_spmd
    nc = _get_program()
    return run_bass_kernel_spmd(
        nc, in_maps, core_ids=list(range(NCORES)), trace=trace)


def kernel(**inputs):
    in_maps = _make_in_maps(**inputs)
    res = _run(in_maps, trace=False)
    return _assemble(res.results)


# revision 10
# speedup vs baseline: 1.5953x; 1.5953x over previous
"""BertSelfAttention Trainium2 kernel.

Full inputs in, full output out. Sharding: 8 cores = (batch b in {0,1}) x
(head-group hg in {0..3}); each core computes 4 heads of one batch and
produces the output feature slice out[b, :, hg*256:(hg+1)*256].

Per-core device program (all cores run the same NEFF, SPMD):
  xT [1024, 2048]      hidden_states[b].T
  QT/KT computed transposed [d, s] (fp32r matmuls), stored fp16 with bias
  V computed [s, d] fp16, rows scaled by exp(mask), plus a per-head
    ones*exp(mask) column so the ctx matmul also yields softmax row sums
  scoresT [k, q] tiles via fp16 matmuls (two heads on the two PE
    row-halves via tile_position)
  exp on ACT directly from PSUM (scale=1/8, bias=-4 folded in)
  ctx[q, d] = expT.T @ [V|em] accumulated over 16 k-tiles, then
    per-partition normalize (reciprocal of row sum) + V-bias add on DVE.

All PE instructions are chained with nosync deps in a hand-balanced
order (scores batches / ctx groups / projection filler) so the PE never
idles long enough for the HAM clock gate to re-throttle it to 1.2 GHz.
"""

import numpy as np

B = 2
S = 2048
H = 1024
NH = 16
HD = 64

NCORES = 8
HPC = 4          # heads per core
DS = HPC * HD    # 256 output dims per core
FT = H // 128    # 8 f-tiles (contraction tiles for projections)
KT = S // 128    # 16 key tiles
ST = S // 128    # 16 s-tiles of V
QB = 4           # q blocks of 512
QBS = 512
VW = HPC * (HD + 1)  # 260: V columns + one em column per head

EXP_BIAS = -4.0  # uniform shift inside exp; cancels in softmax, guards fp16

_CACHE = {}


def _build_program(split_waits=True):
    import concourse.bass as bass
    import concourse.mybir as mybir
    import concourse.tile as tile
    from concourse.vector_clock import ScopedClock

    f32 = mybir.dt.float32
    f32r = mybir.dt.float32r
    f16 = mybir.dt.float16
    AF = mybir.ActivationFunctionType
    OP = mybir.AluOpType

    class SplitDrainTileContext(tile.TileContext):
        """The walrus build here rejects instructions with more than one
        sync wait ("Too many sync wait commands"); hoist excess waits onto
        preceding same-engine NOPs."""

        MAX_WAITS_PER_DRAIN = 1
        split_waits_enabled = True

        def _drain_and_barrier(self, tick_clock, wait_clock):
            drain_inst = self.nc.sync.drain()
            wait_clock.add_sem_waits(
                drain_inst.ins, ScopedClock({None: tick_clock.global_clock})
            )
            self.nc.all_engine_barrier()
            assert self.sems is not None
            popped = self.nc._tile_sem_poison_stack.pop()
            assert popped is self._sem_poison
            self.nc.clear_and_free_semaphores(list(self.sems.allocated().values()))
            self.nc.all_engine_barrier()
            if self.split_waits_enabled:
                self._split_multi_waits()

        def _split_multi_waits(self):
            k = self.MAX_WAITS_PER_DRAIN
            nc = self.nc
            for bb in nc.bb_map.values():
                il = bb.bb.instructions
                new = []
                for inst in il:
                    si = getattr(inst, "sync_info", None)
                    waits = list(si.on_wait) if si is not None and si.on_wait else []
                    if len(waits) > k:
                        for j in range(0, len(waits) - k, k):
                            nop = mybir.InstNoOp(
                                name=nc.get_next_instruction_name(),
                                engine=inst.engine,
                                sync_info=mybir.SyncInfo(
                                    on_wait=waits[j : j + k], on_update=[]
                                ),
                                bass_nofuse=True,
                            )
                            new.append(nop)
                        inst.sync_info = mybir.SyncInfo(
                            on_wait=waits[len(waits) - k :],
                            on_update=list(si.on_update) if si.on_update else [],
                        )
                    new.append(inst)
                il[:] = new

    nc = bass.Bass("TRN2", target_bir_lowering=False, debug=False,
                   num_devices=NCORES)

    xT_d = nc.dram_tensor("xT", [H, S], f16, kind="ExternalInput")
    wqT_d = nc.dram_tensor("wqT", [H, DS], f16, kind="ExternalInput")
    wkT_d = nc.dram_tensor("wkT", [H, DS], f16, kind="ExternalInput")
    wvT_d = nc.dram_tensor("wvT", [H, VW], f16, kind="ExternalInput")
    bq_d = nc.dram_tensor("bq", [2, 128, 1], f32, kind="ExternalInput")
    bk_d = nc.dram_tensor("bk", [2, 128, 1], f32, kind="ExternalInput")
    bvb_d = nc.dram_tensor("bvb", [128, DS], f32, kind="ExternalInput")
    em_d = nc.dram_tensor("em", [128, KT], f32, kind="ExternalInput")
    out_d = nc.dram_tensor("out", [S, DS], f32, kind="ExternalOutput")

    SplitDrainTileContext.split_waits_enabled = split_waits
    with SplitDrainTileContext(nc) as tc:
        from contextlib import ExitStack

        with ExitStack() as ctx:
            const = ctx.enter_context(tc.tile_pool(name="const", bufs=1))
            qk = ctx.enter_context(tc.tile_pool(name="qk", bufs=1))
            vp = ctx.enter_context(tc.tile_pool(name="vp", bufs=1))
            epool = ctx.enter_context(tc.tile_pool(name="epool", bufs=1))
            opool = ctx.enter_context(tc.tile_pool(name="opool", bufs=1))
            rpool = ctx.enter_context(tc.tile_pool(name="rpool", bufs=1))

            # ---- constants ----
            bq_sb = [const.tile([128, 1], f32, tag=f"bq{m}", bufs=1,
                                name=f"bq_sb{m}") for m in range(2)]
            bk_sb = [const.tile([128, 1], f32, tag=f"bk{m}", bufs=1,
                                name=f"bk_sb{m}") for m in range(2)]
            for m in range(2):
                nc.sync.dma_start(bq_sb[m][:], bq_d.ap()[m])
                nc.sync.dma_start(bk_sb[m][:], bk_d.ap()[m])
            bvb_sb = const.tile([128, DS], f32, tag="bvb", bufs=1, name="bvb_sb")
            nc.sync.dma_start(bvb_sb[:], bvb_d.ap())
            em_sb = const.tile([128, KT], f32, tag="em", bufs=1, name="em_sb")
            nc.sync.dma_start(em_sb[:], em_d.ap())
            ebias = const.tile([128, 1], f32, tag="ebias", bufs=1, name="ebias")
            nc.vector.memset(ebias[:], EXP_BIAS)
            # warm the ACT exp table while DMAs run
            warm = const.tile([128, 1], f32, tag="warm", bufs=1, name="warm")
            nc.scalar.activation(warm[:], ebias[:], AF.Exp)

            # ---- persistent activations ----
            qt = [qk.tile([128, S], f16, tag=f"qt{m}", bufs=1, name=f"qt{m}")
                  for m in range(2)]
            kt_sb = [qk.tile([128, S], f16, tag=f"kt{m}", bufs=1, name=f"kt{m}")
                     for m in range(2)]
            vones = [vp.tile([128, VW], f16, tag=f"v{st}", bufs=1,
                             name=f"vones{st}") for st in range(ST)]

            # ---- input DMAs (xT + m0-needed weights first) ----
            xw = ctx.enter_context(tc.tile_pool(name="xw", bufs=1))
            xt = [xw.tile([128, S], f16, tag=f"xt{ft}", bufs=1, name=f"xt{ft}")
                  for ft in range(FT)]
            wq_sb = [xw.tile([128, DS], f16, tag=f"wq{ft}", bufs=1,
                             name=f"wq{ft}") for ft in range(FT)]
            wk_sb = [xw.tile([128, DS], f16, tag=f"wk{ft}", bufs=1,
                             name=f"wk{ft}") for ft in range(FT)]
            wv_sb = [xw.tile([128, VW], f16, tag=f"wv{ft}", bufs=1,
                             name=f"wv{ft}") for ft in range(FT)]
            for ft in range(FT):
                fs = slice(ft * 128, (ft + 1) * 128)
                nc.sync.dma_start(xt[ft][:], xT_d.ap()[fs, :])
                nc.sync.dma_start(wq_sb[ft][:], wqT_d.ap()[fs, :])
                nc.sync.dma_start(wk_sb[ft][:], wkT_d.ap()[fs, :])
            for ft in range(FT):
                fs = slice(ft * 128, (ft + 1) * 128)
                nc.sync.dma_start(wv_sb[ft][:], wvT_d.ap()[fs, :])

            # ---- PSUM pools: proj 1 + scores 2x3 + ctx 1 = 8 banks ----
            ps_pj = ctx.enter_context(
                tc.tile_pool(name="ps_pj", bufs=1, space="PSUM"))
            ps_sc = ctx.enter_context(
                tc.tile_pool(name="ps_sc", bufs=2, space="PSUM"))
            ps_cx = ctx.enter_context(
                tc.tile_pool(name="ps_cx", bufs=1, space="PSUM"))

            mm = nc.tensor.matmul

            # ---- work units (each emits PE work + its evictions) ----
            def qk_proj_block(w_sb, bias_sb, dst, m, nb):
                ns = slice(nb * QBS, (nb + 1) * QBS)
                ps = ps_pj.tile([128, QBS], f32, tag="pj", name="pspj")
                for ft in range(FT):
                    mm(ps[:],
                       w_sb[ft][:, m * 128:(m + 1) * 128],
                       xt[ft][:, ns],
                       start=(ft == 0), stop=(ft == FT - 1))
                nc.vector.tensor_scalar_add(dst[:, ns], ps[:], bias_sb[:])

            def v_proj_block(st):
                ss = slice(st * 128, (st + 1) * 128)
                ps = ps_pj.tile([128, QBS], f32, tag="pj", name="pspjv")
                for ft in range(FT):
                    mm(ps[:, 0:VW],
                       xt[ft][:, ss],
                       wv_sb[ft][:],
                       start=(ft == 0), stop=(ft == FT - 1))
                nc.vector.tensor_scalar_mul(
                    vones[st][:], ps[:, 0:VW], em_sb[:, st:st + 1])
                for hh in range(HPC):
                    c = hh * (HD + 1) + HD
                    nc.vector.tensor_copy(
                        vones[st][:, c:c + 1], em_sb[:, st:st + 1])

            BATCHES = [(0, 3), (3, 3), (6, 3), (9, 3), (12, 3), (15, 1)]

            def scores_batch(hp, qb, eA, eB, k0, nk):
                qs = slice(qb * QBS, (qb + 1) * QBS)
                psA = ps_sc.tile([128, 3 * QBS], f32, tag="sc", name="pscA")
                psB = ps_sc.tile([128, 3 * QBS], f32, tag="sc", name="pscB")
                for j in range(nk):
                    ktile = k0 + j
                    ks = slice(ktile * 128, (ktile + 1) * 128)
                    js = slice(j * QBS, (j + 1) * QBS)
                    mm(psA[:, js], kt_sb[hp][0:64, ks], qt[hp][0:64, qs],
                       tile_position=(0, 0))
                    mm(psB[:, js], kt_sb[hp][64:128, ks], qt[hp][64:128, qs],
                       tile_position=(64, 0))
                w = nk * QBS
                es = slice(k0 * QBS, k0 * QBS + w)
                nc.scalar.activation(eA[:, es], psA[:, 0:w], AF.Exp,
                                     bias=ebias[:], scale=0.125)
                nc.scalar.activation(eB[:, es], psB[:, 0:w], AF.Exp,
                                     bias=ebias[:], scale=0.125)

            def ctx_unit(hp, qb, a, e, ots):
                """One head's 4 q-tiles of ctx for (hp, qb)."""
                hh = 2 * hp + a
                qtile0 = qb * 4
                cpsb = ps_cx.tile([128, 4 * (HD + 1)], f32, tag="cx",
                                  name="cps")
                for qq in range(4):
                    cps = cpsb[:, qq * (HD + 1):(qq + 1) * (HD + 1)]
                    for ktile in range(KT):
                        lo = ktile * QBS + qq * 128
                        mm(cps,
                           e[:, lo:lo + 128],
                           vones[ktile][:, hh * (HD + 1):(hh + 1) * (HD + 1)],
                           start=(ktile == 0), stop=(ktile == KT - 1))
                    if a == 0:
                        ot = opool.tile([128, 128], f32, tag="ot", bufs=4,
                                        name="ot")
                        ots.append(ot)
                    else:
                        ot = ots[qq]
                    r = rpool.tile([128, 1], f32, tag="r", bufs=4, name="r")
                    nc.vector.reciprocal(r[:], cps[:, HD:HD + 1])
                    nc.vector.scalar_tensor_tensor(
                        ot[:, a * 64:(a + 1) * 64],
                        cps[:, 0:HD], r[:],
                        bvb_sb[:, hh * HD:(hh + 1) * HD],
                        op0=OP.mult, op1=OP.add)
                    if a == 1:
                        qt_idx = qtile0 + qq
                        nc.sync.dma_start(
                            out_d.ap()[qt_idx * 128:(qt_idx + 1) * 128,
                                       hp * 128:(hp + 1) * 128],
                            ot[:])

            def ctx_units(prev_state):
                hp, qb, eA, eB = prev_state
                shared_ots = []
                return [
                    lambda: ctx_unit(hp, qb, 0, eA, shared_ots),
                    lambda: ctx_unit(hp, qb, 1, eB, shared_ots),
                ]

            # ---- emission schedule ----
            # m0 Q/K projection, pipelined by 512-col blocks
            for nb in range(QB):
                qk_proj_block(wq_sb, bq_sb[0], qt[0], 0, nb)
                qk_proj_block(wk_sb, bk_sb[0], kt_sb[0], 0, nb)

            # filler units per attention iteration index 0..7
            def m1_block(nb):
                qk_proj_block(wq_sb, bq_sb[1], qt[1], 1, nb)
                qk_proj_block(wk_sb, bk_sb[1], kt_sb[1], 1, nb)

            fillers = {
                0: [lambda st=st: v_proj_block(st) for st in range(8)],
                1: [lambda st=st: v_proj_block(st) for st in range(8, ST)],
                2: [lambda: m1_block(0), lambda: m1_block(1)],
                3: [lambda: m1_block(2), lambda: m1_block(3)],
            }

            prev = None
            for it in range(8):
                hp, qb = divmod(it, QB)
                eA = epool.tile([128, KT * QBS], f16, tag="eA", bufs=3,
                                name="eA")
                eB = epool.tile([128, KT * QBS], f16, tag="eB", bufs=3,
                                name="eB")
                # other units: filler first (V needed by upcoming ctx),
                # then ctx of the previous iteration
                others = list(fillers.get(it, []))
                if prev is not None:
                    others.extend(ctx_units(prev))
                batches = [
                    (lambda k0=k0, nk=nk: scores_batch(hp, qb, eA, eB, k0, nk))
                    for (k0, nk) in BATCHES
                ]
                # two score batches up front so ACT has work, then spread
                # the other units between the remaining batches
                batches[0]()
                batches[1]()
                n_slots = len(batches) - 1
                per = [len(others) // n_slots] * n_slots
                for i in range(len(others) % n_slots):
                    per[i] += 1
                oi = 0
                for si in range(2, len(batches)):
                    batches[si]()
                    for _ in range(per[si - 2]):
                        others[oi]()
                        oi += 1
                for _ in range(per[-1]):
                    others[oi]()
                    oi += 1
                prev = (hp, qb, eA, eB)
            for u in ctx_units(prev):
                u()

    return nc


def _get_program(split_waits=True):
    key = ("nc", split_waits)
    if key not in _CACHE:
        _CACHE[key] = _build_program(split_waits)
    return _CACHE[key]


def _make_in_maps(hidden_states, attention_mask, Wq, bq, Wk, bk, Wv, bv):
    hidden = np.ascontiguousarray(np.asarray(hidden_states, dtype=np.float32))
    mask = np.asarray(attention_mask, dtype=np.float32)
    Wq = np.asarray(Wq, dtype=np.float32)
    Wk = np.asarray(Wk, dtype=np.float32)
    Wv = np.asarray(Wv, dtype=np.float32)
    bq = np.asarray(bq, dtype=np.float32)
    bk = np.asarray(bk, dtype=np.float32)
    bv = np.asarray(bv, dtype=np.float32)

    WqT = Wq.T  # [in, out]
    WkT = Wk.T
    WvT = Wv.T

    in_maps = []
    for c in range(NCORES):
        b, hg = divmod(c, HPC)
        cols = slice(hg * DS, (hg + 1) * DS)
        xT = np.ascontiguousarray(hidden[b].T.astype(np.float16))
        wqT = np.ascontiguousarray(WqT[:, cols].astype(np.float16))
        wkT = np.ascontiguousarray(WkT[:, cols].astype(np.float16))
        wv_base = WvT[:, cols]
        wvT = np.zeros((H, VW), np.float16)
        for hh in range(HPC):
            wvT[:, hh * (HD + 1):hh * (HD + 1) + HD] = \
                wv_base[:, hh * HD:(hh + 1) * HD]
        bq_c = np.ascontiguousarray(bq[cols].reshape(2, 128, 1))
        bk_c = np.ascontiguousarray(bk[cols].reshape(2, 128, 1))
        bvb = np.ascontiguousarray(np.tile(bv[cols][None, :], (128, 1)))
        em = np.ascontiguousarray(
            np.exp(mask[b, 0, 0, :]).reshape(KT, 128).T.astype(np.float32))
        in_maps.append({
            "xT": xT, "wqT": wqT, "wkT": wkT, "wvT": wvT,
            "bq": bq_c, "bk": bk_c, "bvb": bvb, "em": em,
        })
    return in_maps


def _assemble(results):
    out = np.empty((B, S, H), np.float32)
    for c in range(NCORES):
        b, hg = divmod(c, HPC)
        out[b][:, hg * DS:(hg + 1) * DS] = results[c]["out"]
    return out


def _run(in_maps, trace=False):
    from concourse.bass_utils import run_bass_kernel_spmd
    nc = _get_program()
    return run_bass_kernel_spmd(
        nc, in_maps, core_ids=list(range(NCORES)), trace=trace)


def kernel(**inputs):
    in_maps = _make_in_maps(**inputs)
    res = _run(in_maps, trace=False)
    return _assemble(res.results)
